# revision 8
# baseline (speedup 1.0000x reference)
"""MemNet (6-hop memory network) Trainium2 kernel — 8-core data parallel.

Strategy:
  - Shard batch (B=256) across 8 cores, 32 slots each. Host sorts batch rows
    by context_len descending and deals them round-robin so the 8 cores have
    near-identical length profiles; the program is specialized to the per-slot
    max chunk count (1 or 2 l-chunks of 128).
  - Embedding rows are fetched with indirect-DMA row gathers from a host-
    augmented table [VOCAB+1, 304]: cols 0..299 = embedding, col 300 =
    emb @ w_att[:D] (the hop-invariant score term), row VOCAB = zeros
    (out-of-length ids redirect there, keeping everything NaN-free and exact).
  - Per hop: linear/proj matmuls from a transposed vec; scores via fused
    ACT ops (tanh/exp with per-partition bias, accum row-sum); softmax
    denominator corrected for the uniform masked tail; attention as per-slot
    thin matmuls with block-diagonal masked lhsT accumulating straight into
    the linear-out PSUM tile. v_loc is folded into the attention weights.
  - All matmul operands are float32r (~14 mantissa bits) for 1 cycle/row.
"""

import sys

sys.path.insert(0, "/opt/trn_rl_repo")

import numpy as np

import concourse.mybir as mybir
import concourse.tile as tile
from concourse.bacc import Bacc
from concourse.bass import AP, IndirectOffsetOnAxis
from concourse.bass_utils import run_bass_kernel_spmd
from concourse.masks import make_identity

VOCAB, D, L, LT, B = 50000, 300, 256, 8, 256
N_HOPS, N_CLASSES = 6, 3
NCORES = 8
BL = B // NCORES  # 32 slots per core
W = 304  # augmented row width: 300 emb + 1 ewa + 3 pad
P = 128
F32 = mybir.dt.float32
F32R = mybir.dt.float32r
I32 = mybir.dt.int32


def _diag_ap(t_ap, lc, n):
    """[128, n] stride-(n+1) diagonal view into block lc of [128, 2, n, n]."""
    return AP(t_ap.tensor, t_ap.offset + lc * n * n, [t_ap.ap[0], [n + 1, n]])


def _block_ap(t_ap, lc, slot, n):
    """[128, n] block (lc, slot) of [128, 2, n, n]."""
    return AP(
        t_ap.tensor, t_ap.offset + (lc * n + slot) * n, [t_ap.ap[0], [1, n]]
    )


def build_program(n2, b_att, reps=1, debug=False):
    """Build the SPMD program. n2 = number of slots with 2 l-chunks
    (slots 0..n2-1), b_att = scalar attention bias baked in."""
    nch = BL + n2  # total context gather chunks
    nc = Bacc("TRN2", target_bir_lowering=False, debug=False, num_devices=NCORES)

    dt_emb = nc.dram_tensor("emb", (VOCAB + 1, W), F32, kind="ExternalInput").ap()
    dt_idx = nc.dram_tensor("idx_ctx", (P, nch), I32, kind="ExternalInput").ap()
    dt_idxt = nc.dram_tensor("idx_tgt", (P, 2), I32, kind="ExternalInput").ap()
    dt_tw = nc.dram_tensor("tw", (P, 2, BL), F32, kind="ExternalInput").ap()
    dt_wlin = nc.dram_tensor("wlin", (P, 3, D + 4), F32, kind="ExternalInput").ap()
    dt_wout = nc.dram_tensor("wout", (P, 3, 8), F32, kind="ExternalInput").ap()
    dt_blin = nc.dram_tensor("blin", (BL, D), F32, kind="ExternalInput").ap()
    dt_bout = nc.dram_tensor("bout", (BL, N_CLASSES), F32, kind="ExternalInput").ap()
    # aux cols: 0=len_f, 1=loc_f, 2=nlen_f(256-len), 3=rlen_f(1/len)
    dt_aux = nc.dram_tensor("aux", (BL, 4), F32, kind="ExternalInput").ap()
    do_out = nc.dram_tensor("logits", (BL, N_CLASSES), F32, kind="ExternalOutput").ap()
    dbg = {}
    if debug:
        for nm, shp in (
            [("d_vaspect", (BL, D)), ("d_s0", (BL, L)), ("d_vloc", (BL, L)),
             ("d_scor", (BL, L)), ("d_en", (BL, L)), ("d_summ", (BL, 1))]
            + [(f"d_vec{h}", (BL, D)) for h in range(N_HOPS)]
        ):
            dbg[nm] = nc.dram_tensor(nm, shp, F32, kind="ExternalOutput").ap()

    with tile.TileContext(nc) as tc:
        with tc.tile_pool(name="sb", bufs=1) as sb, tc.tile_pool(
            name="ps", bufs=1, space="PSUM"
        ) as ps:
            # ---- persistent weights / constants (loaded once) ----
            ident = sb.tile([P, P], F32)
            make_identity(nc, ident[:])
            wlin = sb.tile([P, 3, D + 4], F32)
            nc.sync.dma_start(wlin[:], dt_wlin[:])
            wout = sb.tile([P, 3, 8], F32)
            nc.sync.dma_start(wout[:], dt_wout[:])
            tw = sb.tile([P, 2, BL], F32R)
            nc.gpsimd.dma_start(tw[:], dt_tw[:].bitcast(F32R))
            blin = sb.tile([BL, D], F32)
            nc.sync.dma_start(blin[:], dt_blin[:])
            bout = sb.tile([BL, N_CLASSES], F32)
            nc.sync.dma_start(bout[:], dt_bout[:])
            aux = sb.tile([BL, 4], F32)
            nc.sync.dma_start(aux[:], dt_aux[:])
            idx = sb.tile([P, nch], I32)
            nc.sync.dma_start(idx[:], dt_idx[:])
            idxt = sb.tile([P, 2], I32)
            nc.sync.dma_start(idxt[:], dt_idxt[:])
            neg1 = sb.tile([BL, 1], F32)
            nc.vector.memset(neg1[:], -1.0)
            zero128 = nc.const_aps.tensor(0.0, [P, 1])

            # iota / mask / vloc precompute (depends only on aux)
            iota_f = sb.tile([BL, L], F32)
            nc.gpsimd.iota(
                iota_f[:],
                pattern=[[1, L]],
                base=0,
                channel_multiplier=0,
                allow_small_or_imprecise_dtypes=True,
            )
            mask = sb.tile([BL, L], F32)
            nc.vector.tensor_scalar(
                out=mask[:], in0=iota_f[:], scalar1=aux[:, 0:1], scalar2=None,
                op0=mybir.AluOpType.is_lt,
            )
            dist = sb.tile([BL, L], F32)
            nc.vector.tensor_scalar(
                out=dist[:], in0=iota_f[:], scalar1=aux[:, 1:2], scalar2=None,
                op0=mybir.AluOpType.subtract,
            )
            nc.scalar.activation(dist[:], dist[:], mybir.ActivationFunctionType.Abs)
            vraw = sb.tile([BL, L], F32)
            # vraw = 1 - dist * rlen  (Copy: out = in*scale + bias)
            negr = sb.tile([BL, 1], F32)
            nc.vector.tensor_scalar(
                out=negr[:], in0=aux[:, 3:4], scalar1=-1.0, scalar2=None,
                op0=mybir.AluOpType.mult,
            )
            nc.scalar.activation(
                vraw[:], dist[:], mybir.ActivationFunctionType.Copy,
                bias=1.0, scale=negr[:],
            )
            vloc = sb.tile([BL, L], F32)
            nc.vector.tensor_tensor(
                out=vloc[:], in0=vraw[:], in1=mask[:], op=mybir.AluOpType.mult
            )
            # vlocT [128, 2, 32]
            vlocT = sb.tile([P, 2, BL], F32)
            for lc in range(2):
                vT_ps = ps.tile([P, BL], F32, space="PSUM", tag="tp", bufs=4)
                nc.tensor.transpose(
                    out=vT_ps[:], in_=vloc[:, lc * P : (lc + 1) * P],
                    identity=ident[0:BL, 0:BL],
                )
                nc.vector.tensor_copy(vlocT[:, lc, :], vT_ps[:])
            if debug:
                nc.sync.dma_start(dbg["d_vloc"][:], vloc[:])

            # big zeroed block-diag alpha tile [128, 2, 32, 32]
            abig = sb.tile([P, 2, BL, BL], F32R)
            abig_flat = AP(
                abig[:].tensor, abig[:].offset, [abig[:].ap[0], [1, 2 * BL * BL]]
            )
            nc.vector.tensor_copy(abig_flat, zero128.to_broadcast([P, 2 * BL * BL]))

            for rep in range(reps):
                # ---- gathers ----
                emt = sb.tile([P, 2, W], F32R, tag="emt")
                for ch in range(2):
                    nc.gpsimd.indirect_dma_start(
                        out=emt[:, ch, :],
                        out_offset=None,
                        in_=dt_emb[:].bitcast(F32R),
                        in_offset=IndirectOffsetOnAxis(ap=idxt[:, ch : ch + 1], axis=0),
                    )
                em = sb.tile([P, nch, W], F32R, tag="em")
                for c in range(nch):
                    nc.gpsimd.indirect_dma_start(
                        out=em[:, c, :],
                        out_offset=None,
                        in_=dt_emb[:].bitcast(F32R),
                        in_offset=IndirectOffsetOnAxis(ap=idx[:, c : c + 1], axis=0),
                    )

                # ---- v_aspect -> vec ----
                vec = sb.tile([BL, D], F32, tag="vec")
                va_ps = ps.tile([BL, D], F32, space="PSUM", tag="tp", bufs=4)
                for ch in range(2):
                    nc.tensor.matmul(
                        out=va_ps[:],
                        lhsT=tw[:, ch, :],
                        rhs=emt[:, ch, 0:D],
                        start=(ch == 0), stop=(ch == 1), skip_group_check=True,
                    )
                nc.vector.tensor_copy(vec[:], va_ps[:])
                if debug:
                    nc.sync.dma_start(dbg["d_vaspect"][:], vec[:])

                # ---- s0 assembly from gathered col 300 ----
                s0 = sb.tile([BL, L], F32, tag="s0")
                nc.vector.memset(s0[:], 0.0)
                em_ap = em[:]
                # strided view of ewa column: [128, nchunk] stride W
                for lc, (c0, cn) in enumerate([(0, BL), (BL, n2)]):
                    if cn == 0:
                        continue
                    col = AP(
                        em_ap.tensor, em_ap.offset + c0 * W + D, [em_ap.ap[0], [W, cn]]
                    )
                    s0_ps = ps.tile([BL, P], F32, space="PSUM", tag="tp", bufs=4)
                    nc.tensor.transpose(
                        out=s0_ps[0:cn, :],
                        in_=col.bitcast(F32),
                        identity=ident[0:P, 0:P],
                    )
                    nc.vector.tensor_copy(
                        s0[0:cn, lc * P : (lc + 1) * P], s0_ps[0:cn, :]
                    )
                if debug:
                    nc.sync.dma_start(dbg["d_s0"][:], s0[:])
                s0v = sb.tile([BL, L], F32, tag="s0v")
                nc.vector.tensor_tensor(
                    out=s0v[:], in0=s0[:], in1=vloc[:], op=mybir.AluOpType.mult
                )

                # ---- hops ----
                for h in range(N_HOPS):
                    # vecT [128, 3, 32]
                    vecT = sb.tile([P, 3, BL], F32, tag="vecT")
                    for ch in range(3):
                        k = min(P, D - ch * P)
                        vT = ps.tile([P, BL], F32, space="PSUM", tag="tp", bufs=4)
                        nc.tensor.transpose(
                            out=vT[0:k, :],
                            in_=vec[:, ch * P : ch * P + k],
                            identity=ident[0:BL, 0:BL],
                        )
                        nc.vector.tensor_copy(vecT[0:k, ch, :], vT[0:k, :])
                    # linear_out (+proj in col 300) into acc psum
                    acc = ps.tile([BL, D + 4], F32, space="PSUM", tag="acc", bufs=2)
                    for ch in range(3):
                        k = min(P, D - ch * P)
                        nc.tensor.matmul(
                            out=acc[:],
                            lhsT=vecT[0:k, ch, :],
                            rhs=wlin[0:k, ch, :],
                            start=(ch == 0), stop=False, skip_group_check=True,
                        )
                    projb = sb.tile([BL, 1], F32, tag="projb")
                    nc.scalar.activation(
                        projb[:], acc[:, D : D + 1], mybir.ActivationFunctionType.Copy,
                        bias=float(b_att),
                    )
                    # scores -> e (exp(tanh(s0v + projb) - 1)), accum row sums
                    scor = sb.tile([BL, L], F32, tag="scor")
                    nc.scalar.activation(
                        scor[:], s0v[:], mybir.ActivationFunctionType.Tanh,
                        bias=projb[:],
                    )
                    e = sb.tile([BL, L], F32, tag="e")
                    nc.scalar.activation(
                        e[:], scor[:], mybir.ActivationFunctionType.Exp,
                        bias=neg1[:],
                    )
                    em_ = sb.tile([BL, L], F32, tag="em_")
                    nc.vector.tensor_tensor(
                        out=em_[:], in0=e[:], in1=mask[:], op=mybir.AluOpType.mult
                    )
                    summ = sb.tile([BL, 1], F32, tag="summ")
                    nc.vector.reduce_sum(
                        out=summ[:], in_=em_[:], axis=mybir.AxisListType.X
                    )
                    recip0 = sb.tile([BL, 1], F32, tag="recip0")
                    nc.vector.reciprocal(recip0[:], summ[:])
                    rtmp = sb.tile([BL, 1], F32, tag="rtmp")
                    nc.vector.tensor_tensor(
                        out=rtmp[:], in0=summ[:], in1=recip0[:],
                        op=mybir.AluOpType.mult,
                    )
                    nc.vector.tensor_scalar(
                        out=rtmp[:], in0=rtmp[:], scalar1=2.0, scalar2=-1.0,
                        op0=mybir.AluOpType.subtract, op1=mybir.AluOpType.mult,
                    )
                    recip = sb.tile([BL, 1], F32, tag="recip")
                    nc.vector.tensor_tensor(
                        out=recip[:], in0=recip0[:], in1=rtmp[:],
                        op=mybir.AluOpType.mult,
                    )
                    en = sb.tile([BL, L], F32, tag="en")
                    nc.vector.tensor_scalar(
                        out=en[:], in0=em_[:], scalar1=recip[:], scalar2=None,
                        op0=mybir.AluOpType.mult,
                    )
                    # alphaT diag writes: abig[p, lc, s, s] = enT * vlocT
                    for lc in range(2):
                        eT_ps = ps.tile([P, BL], F32, space="PSUM", tag="tp", bufs=4)
                        nc.tensor.transpose(
                            out=eT_ps[:], in_=en[:, lc * P : (lc + 1) * P],
                            identity=ident[0:BL, 0:BL],
                        )
                        nc.vector.tensor_tensor(
                            out=_diag_ap(abig[:], lc, BL),
                            in0=eT_ps[:], in1=vlocT[:, lc, :],
                            op=mybir.AluOpType.mult,
                        )
                    # attention: accumulate into acc
                    for c in range(nch):
                        slot, lc = (c, 0) if c < BL else (c - BL, 1)
                        nc.tensor.matmul(
                            out=acc[:, 0:D],
                            lhsT=_block_ap(abig[:], lc, slot, BL),
                            rhs=em[:, c, 0:D],
                            start=False, stop=(c == nch - 1), skip_group_check=True,
                        )
                    # vec = acc + b_lin
                    vec = sb.tile([BL, D], F32, tag="vec")
                    nc.vector.tensor_tensor(
                        out=vec[:], in0=acc[:, 0:D], in1=blin[:], op=mybir.AluOpType.add
                    )
                    if debug:
                        nc.sync.dma_start(dbg[f"d_vec{h}"][:], vec[:])
                        if h == 0:
                            nc.sync.dma_start(dbg["d_scor"][:], scor[:])
                            nc.sync.dma_start(dbg["d_en"][:], en[:])
                            nc.sync.dma_start(dbg["d_summ"][:], summ[:])

                # ---- output ----
                vecT = sb.tile([P, 3, BL], F32, tag="vecT")
                for ch in range(3):
                    k = min(P, D - ch * P)
                    vT = ps.tile([P, BL], F32, space="PSUM", tag="tp", bufs=4)
                    nc.tensor.transpose(
                        out=vT[0:k, :],
                        in_=vec[:, ch * P : ch * P + k],
                        identity=ident[0:BL, 0:BL],
                    )
                    nc.vector.tensor_copy(vecT[0:k, ch, :], vT[0:k, :])
                lg_ps = ps.tile([BL, 8], F32, space="PSUM", tag="tp", bufs=4)
                for ch in range(3):
                    k = min(P, D - ch * P)
                    nc.tensor.matmul(
                        out=lg_ps[:],
                        lhsT=vecT[0:k, ch, :],
                        rhs=wout[0:k, ch, :],
                        start=(ch == 0), stop=(ch == 2), skip_group_check=True,
                    )
                logits = sb.tile([BL, N_CLASSES], F32, tag="logits")
                nc.vector.tensor_tensor(
                    out=logits[:], in0=lg_ps[:, 0:N_CLASSES], in1=bout[:],
                    op=mybir.AluOpType.add,
                )
                nc.sync.dma_start(do_out[:], logits[:])

    nc.finalize()
    return nc


def prep_inputs(embeddings, w_lin, b_lin, w_att, b_att, w_out, b_out,
                context_ids, context_len, target_ids, target_len, target_loc):
    """Host-side prep: batch assignment, packed indices/weights, in_maps."""
    embeddings = np.ascontiguousarray(embeddings, np.float32)
    w_lin = np.asarray(w_lin, np.float32)
    b_lin = np.asarray(b_lin, np.float32)
    w_att = np.asarray(w_att, np.float32)
    b_att = float(np.asarray(b_att))
    w_out = np.asarray(w_out, np.float32)
    b_out = np.asarray(b_out, np.float32)
    context_ids = np.asarray(context_ids, np.int64)
    context_len = np.asarray(context_len, np.int64)
    target_ids = np.asarray(target_ids, np.int64)
    target_len = np.asarray(target_len, np.int64)
    target_loc = np.asarray(target_loc, np.int64)

    # sort descending by len, deal round-robin: slot i core k <- order[i*8+k]
    order = np.argsort(-context_len, kind="stable")
    assign = order.reshape(BL, NCORES)  # [slot, core]
    slot_max = context_len[assign].max(axis=1)  # [BL]
    nchunks = np.where(slot_max > P, 2, 1)
    n2 = int((nchunks == 2).sum())
    nch = BL + n2

    # augmented embedding table
    ewa = embeddings @ w_att[:D]
    emb_aug = np.zeros((VOCAB + 1, W), np.float32)
    emb_aug[:VOCAB, :D] = embeddings
    emb_aug[:VOCAB, D] = ewa

    # packed weights (chunked on d, [128, 3, ...])
    def pack_d(x):  # x [300, ...] -> [128, 3, ...]
        out = np.zeros((P, 3) + x.shape[1:], np.float32)
        for ch in range(3):
            k = min(P, D - ch * P)
            out[:k, ch] = x[ch * P : ch * P + k]
        return out

    wlin_pk = pack_d(np.concatenate(
        [w_lin, w_att[D:, None], np.zeros((D, 3), np.float32)], axis=1))
    wout_pk = pack_d(np.concatenate(
        [w_out, np.zeros((D, 8 - N_CLASSES), np.float32)], axis=1))
    blin_rep = np.tile(b_lin[None, :], (BL, 1))
    bout_rep = np.tile(b_out[None, :], (BL, 1))

    in_maps = []
    for k in range(NCORES):
        bidx = assign[:, k]  # batch row per slot
        lens = context_len[bidx]
        locs = target_loc[bidx]
        tlens = target_len[bidx]

        idx_ctx = np.full((P, nch), VOCAB, np.int32)
        for c in range(nch):
            slot, lc = (c, 0) if c < BL else (c - BL, 1)
            ln = int(lens[slot])
            base = lc * P
            nvalid = min(max(ln - base, 0), P)
            if nvalid > 0:
                idx_ctx[:nvalid, c] = context_ids[bidx[slot], base : base + nvalid]

        idx_tgt = np.full((P, 2), VOCAB, np.int32)
        for ch in range(2):
            for j in range(16):
                slot = ch * 16 + j
                tl = int(tlens[slot])
                idx_tgt[j * 8 : j * 8 + tl, ch] = target_ids[bidx[slot], :tl]

        tw = np.zeros((P, 2, BL), np.float32)
        for ch in range(2):
            for j in range(16):
                tw[j * 8 : (j + 1) * 8, ch, ch * 16 + j] = 1.0 / float(tlens[ch * 16 + j])

        aux = np.stack(
            [
                lens.astype(np.float32),
                locs.astype(np.float32),
                (L - lens).astype(np.float32),
                1.0 / lens.astype(np.float32),
            ],
            axis=1,
        )
        in_maps.append(
            {
                "emb": emb_aug,
                "idx_ctx": idx_ctx,
                "idx_tgt": idx_tgt,
                "tw": tw,
                "wlin": wlin_pk,
                "wout": wout_pk,
                "blin": blin_rep,
                "bout": bout_rep,
                "aux": aux,
            }
        )
    return in_maps, assign, n2, b_att


def kernel(**inputs) -> np.ndarray:
    in_maps, assign, n2, b_att = prep_inputs(**inputs)
    nc = build_program(n2, b_att)
    res = run_bass_kernel_spmd(nc, in_maps, core_ids=list(range(NCORES)))
    out = np.zeros((B, N_CLASSES), np.float32)
    for k in range(NCORES):
        out[assign[:, k]] = res.results[k]["logits"]
    return out


# revision 12
# speedup vs baseline: 1.2203x; 1.2203x over previous
"""MemNet (6-hop memory network) Trainium2 kernel — 8-core data parallel.

Strategy:
  - Shard batch (B=256) across 8 cores, 32 slots each. Host sorts batch rows
    by context_len descending and deals them round-robin so the 8 cores have
    near-identical length profiles; the program is specialized to the per-slot
    max chunk count (1 or 2 l-chunks of 128).
  - Embedding rows are fetched with indirect-DMA row gathers from a host-
    augmented table [VOCAB+1, 304]: cols 0..299 = embedding, col 300 =
    emb @ w_att[:D] (the hop-invariant score term), row VOCAB = zeros
    (out-of-length ids redirect there, keeping everything NaN-free and exact).
  - Per hop: linear/proj matmuls from a transposed vec; scores via fused
    ACT ops (tanh/exp with per-partition bias, accum row-sum); softmax
    denominator corrected for the uniform masked tail; attention as per-slot
    thin matmuls with block-diagonal masked lhsT accumulating straight into
    the linear-out PSUM tile. v_loc is folded into the attention weights.
  - All matmul operands are float32r (~14 mantissa bits) for 1 cycle/row.
"""

import sys

sys.path.insert(0, "/opt/trn_rl_repo")

import numpy as np

import concourse.mybir as mybir
import concourse.tile as tile
from concourse.bacc import Bacc
from concourse.bass import AP, IndirectOffsetOnAxis
from concourse.bass_utils import run_bass_kernel_spmd
from concourse.masks import make_identity

VOCAB, D, L, LT, B = 50000, 300, 256, 8, 256
N_HOPS, N_CLASSES = 6, 3
NCORES = 8
BL = B // NCORES  # 32 slots per core
W = 304  # augmented row width: 300 emb + 1 ewa + 3 pad
P = 128
F32 = mybir.dt.float32
F32R = mybir.dt.float32r
I32 = mybir.dt.int32


def _diag_ap(t_ap, lc, n):
    """[128, n] stride-(n+1) diagonal view into block lc of [128, 2, n, n]."""
    return AP(t_ap.tensor, t_ap.offset + lc * n * n, [t_ap.ap[0], [n + 1, n]])


def _block_ap(t_ap, lc, slot, n):
    """[128, n] block (lc, slot) of [128, 2, n, n]."""
    return AP(
        t_ap.tensor, t_ap.offset + (lc * n + slot) * n, [t_ap.ap[0], [1, n]]
    )


def build_program(n2, b_att, chunk_rows, reps=1, debug=False, only_gather=False, skip_attn=False, nq=4):
    """Build the SPMD program. n2 = number of slots with 2 l-chunks
    (slots 0..n2-1), b_att = attention bias baked in, chunk_rows[c] = rows
    gathered/streamed for context chunk c (1..128, from per-slot max len)."""
    nch = BL + n2  # total context gather chunks
    nc = Bacc("TRN2", target_bir_lowering=False, debug=False, num_devices=NCORES,
              num_swdge_queues=nq)

    dt_emb = nc.dram_tensor("emb", (VOCAB + 1, W), F32, kind="ExternalInput").ap()
    dt_idx = nc.dram_tensor("idx_ctx", (P, nch), I32, kind="ExternalInput").ap()
    dt_idxt = nc.dram_tensor("idx_tgt", (P, 2), I32, kind="ExternalInput").ap()
    dt_tw = nc.dram_tensor("tw", (P, 2, BL), F32, kind="ExternalInput").ap()
    dt_wlin = nc.dram_tensor("wlin", (P, 3, D + 4), F32, kind="ExternalInput").ap()
    dt_wout = nc.dram_tensor("wout", (P, 3, 8), F32, kind="ExternalInput").ap()
    dt_blin = nc.dram_tensor("blin", (BL, D), F32, kind="ExternalInput").ap()
    dt_bout = nc.dram_tensor("bout", (BL, N_CLASSES), F32, kind="ExternalInput").ap()
    # aux cols: 0=len_f, 1=loc_f, 2=nlen_f(256-len), 3=rlen_f(1/len)
    dt_aux = nc.dram_tensor("aux", (BL, 4), F32, kind="ExternalInput").ap()
    dt_s0 = nc.dram_tensor("s0i", (BL, L), F32, kind="ExternalInput").ap()
    do_out = nc.dram_tensor("logits", (BL, N_CLASSES), F32, kind="ExternalOutput").ap()
    dbg = {}
    if debug:
        for nm, shp in (
            [("d_vaspect", (BL, D)), ("d_s0", (BL, L)), ("d_vloc", (BL, L)),
             ("d_scor", (BL, L)), ("d_en", (BL, L)), ("d_summ", (BL, 1))]
            + [(f"d_vec{h}", (BL, D)) for h in range(N_HOPS)]
        ):
            dbg[nm] = nc.dram_tensor(nm, shp, F32, kind="ExternalOutput").ap()

    with tile.TileContext(nc) as tc:
        with tc.tile_pool(name="sb", bufs=1) as sb, tc.tile_pool(
            name="ps", bufs=1, space="PSUM"
        ) as ps:
            # ---- persistent weights / constants (loaded once) ----
            ident = sb.tile([P, P], F32)
            make_identity(nc, ident[:])
            wlin = sb.tile([P, 3, D + 4], F32)
            nc.sync.dma_start(wlin[:], dt_wlin[:])
            wout = sb.tile([P, 3, 8], F32)
            nc.sync.dma_start(wout[:], dt_wout[:])
            tw = sb.tile([P, 2, BL], F32)
            nc.sync.dma_start(tw[:], dt_tw[:])
            blin = sb.tile([BL, D], F32)
            nc.sync.dma_start(blin[:], dt_blin[:])
            bout = sb.tile([BL, N_CLASSES], F32)
            nc.sync.dma_start(bout[:], dt_bout[:])
            aux = sb.tile([BL, 4], F32)
            nc.sync.dma_start(aux[:], dt_aux[:])
            idx = sb.tile([P, nch], I32)
            nc.sync.dma_start(idx[:], dt_idx[:])
            idxt = sb.tile([P, 2], I32)
            nc.sync.dma_start(idxt[:], dt_idxt[:])
            s0 = sb.tile([BL, L], F32)
            nc.sync.dma_start(s0[:], dt_s0[:])
            neg1 = sb.tile([BL, 1], F32)
            nc.vector.memset(neg1[:], -1.0)
            zero128 = nc.const_aps.tensor(0.0, [P, 1])

            # iota / mask / vloc precompute (depends only on aux)
            iota_f = sb.tile([BL, L], F32)
            nc.gpsimd.iota(
                iota_f[:],
                pattern=[[1, L]],
                base=0,
                channel_multiplier=0,
                allow_small_or_imprecise_dtypes=True,
            )
            mask = sb.tile([BL, L], F32)
            nc.vector.tensor_scalar(
                out=mask[:], in0=iota_f[:], scalar1=aux[:, 0:1], scalar2=None,
                op0=mybir.AluOpType.is_lt,
            )
            dist = sb.tile([BL, L], F32)
            nc.vector.tensor_scalar(
                out=dist[:], in0=iota_f[:], scalar1=aux[:, 1:2], scalar2=None,
                op0=mybir.AluOpType.subtract,
            )
            nc.scalar.activation(dist[:], dist[:], mybir.ActivationFunctionType.Abs)
            vraw = sb.tile([BL, L], F32)
            # vraw = 1 - dist * rlen  (Copy: out = in*scale + bias)
            negr = sb.tile([BL, 1], F32)
            nc.vector.tensor_scalar(
                out=negr[:], in0=aux[:, 3:4], scalar1=-1.0, scalar2=None,
                op0=mybir.AluOpType.mult,
            )
            nc.scalar.activation(
                vraw[:], dist[:], mybir.ActivationFunctionType.Copy,
                bias=1.0, scale=negr[:],
            )
            vloc = sb.tile([BL, L], F32)
            nc.vector.tensor_tensor(
                out=vloc[:], in0=vraw[:], in1=mask[:], op=mybir.AluOpType.mult
            )
            # vlocT [128, 2, 32]
            vlocT = sb.tile([P, 2, BL], F32)
            for lc in range(2):
                vT_ps = ps.tile([P, BL], F32, space="PSUM", tag="tp", bufs=4)
                nc.tensor.transpose(
                    out=vT_ps[:], in_=vloc[:, lc * P : (lc + 1) * P],
                    identity=ident[0:BL, 0:BL],
                )
                nc.vector.tensor_copy(vlocT[:, lc, :], vT_ps[:])
            if debug:
                nc.sync.dma_start(dbg["d_vloc"][:], vloc[:])

            # big zeroed block-diag alpha tile [128, 2, 32, 32]
            abig = sb.tile([P, 2, BL, BL], F32R)
            abig_flat = AP(
                abig[:].tensor, abig[:].offset, [abig[:].ap[0], [1, 2 * BL * BL]]
            )
            nc.vector.tensor_copy(abig_flat, zero128.to_broadcast([P, 2 * BL * BL]))

            for rep in range(reps):
                # ---- gathers ----
                emt = sb.tile([P, 2, W], F32, tag="emt")
                for ch in range(2):
                    gi = nc.gpsimd.indirect_dma_start(
                        out=emt[:, ch, :],
                        out_offset=None,
                        in_=dt_emb[:],
                        in_offset=IndirectOffsetOnAxis(ap=idxt[:, ch : ch + 1], axis=0),
                    )
                    if nq > 1:
                        gi.ins.queue = f"qPoolDynamic{ch % nq or ''}"
                em = sb.tile([P, nch, W], F32R, tag="em")
                for c in range(nch):
                    r = chunk_rows[c]
                    gi = nc.gpsimd.indirect_dma_start(
                        out=em[0:r, c, :],
                        out_offset=None,
                        in_=dt_emb[:].bitcast(F32R),
                        in_offset=IndirectOffsetOnAxis(ap=idx[0:r, c : c + 1], axis=0),
                    )
                    if nq > 1:
                        gi.ins.queue = f"qPoolDynamic{c % nq or ''}"

                if only_gather:
                    # consume gathers so they aren't dead-code'd: one reduce
                    gsink = sb.tile([P, 1], F32, tag="gsink")
                    nc.vector.reduce_sum(
                        out=gsink[:], in_=em[:, :, 0].bitcast(F32),
                        axis=mybir.AxisListType.X,
                    )
                    nc.sync.dma_start(do_out[0:1, 0:1], gsink[0:1, :])
                    continue
                # ---- v_aspect -> vec ----
                vec = sb.tile([BL, D], F32, tag="vec")
                va_ps = ps.tile([BL, D], F32, space="PSUM", tag="tp", bufs=4)
                for ch in range(2):
                    nc.tensor.matmul(
                        out=va_ps[:],
                        lhsT=tw[:, ch, :],
                        rhs=emt[:, ch, 0:D],
                        start=(ch == 0), stop=(ch == 1), skip_group_check=True,
                    )
                nc.vector.tensor_copy(vec[:], va_ps[:])
                if debug:
                    nc.sync.dma_start(dbg["d_vaspect"][:], vec[:])

                if debug:
                    nc.sync.dma_start(dbg["d_s0"][:], s0[:])
                s0v = sb.tile([BL, L], F32, tag="s0v")
                nc.vector.tensor_tensor(
                    out=s0v[:], in0=s0[:], in1=vloc[:], op=mybir.AluOpType.mult
                )

                # ---- hops ----
                for h in range(N_HOPS):
                    # vecT [128, 3, 32]
                    vecT = sb.tile([P, 3, BL], F32, tag="vecT")
                    for ch in range(3):
                        k = min(P, D - ch * P)
                        vT = ps.tile([P, BL], F32, space="PSUM", tag="tp", bufs=4)
                        nc.tensor.transpose(
                            out=vT[0:k, :],
                            in_=vec[:, ch * P : ch * P + k],
                            identity=ident[0:BL, 0:BL],
                        )
                        nc.vector.tensor_copy(vecT[0:k, ch, :], vT[0:k, :])
                    # linear_out (+proj in col 300) into acc psum
                    acc = ps.tile([BL, D + 4], F32, space="PSUM", tag="acc", bufs=2)
                    for ch in range(3):
                        k = min(P, D - ch * P)
                        nc.tensor.matmul(
                            out=acc[:],
                            lhsT=vecT[0:k, ch, :],
                            rhs=wlin[0:k, ch, :],
                            start=(ch == 0), stop=False, skip_group_check=True,
                        )
                    projb = sb.tile([BL, 1], F32, tag="projb")
                    nc.scalar.activation(
                        projb[:], acc[:, D : D + 1], mybir.ActivationFunctionType.Copy,
                        bias=float(b_att),
                    )
                    # scores -> e (exp(tanh(s0v + projb) - 1)), accum row sums
                    scor = sb.tile([BL, L], F32, tag="scor")
                    nc.scalar.activation(
                        scor[:], s0v[:], mybir.ActivationFunctionType.Tanh,
                        bias=projb[:],
                    )
                    e = sb.tile([BL, L], F32, tag="e")
                    nc.scalar.activation(
                        e[:], scor[:], mybir.ActivationFunctionType.Exp,
                        bias=neg1[:],
                    )
                    em_ = sb.tile([BL, L], F32, tag="em_")
                    nc.vector.tensor_tensor(
                        out=em_[:], in0=e[:], in1=mask[:], op=mybir.AluOpType.mult
                    )
                    summ = sb.tile([BL, 1], F32, tag="summ")
                    nc.vector.reduce_sum(
                        out=summ[:], in_=em_[:], axis=mybir.AxisListType.X
                    )
                    recip0 = sb.tile([BL, 1], F32, tag="recip0")
                    nc.vector.reciprocal(recip0[:], summ[:])
                    rtmp = sb.tile([BL, 1], F32, tag="rtmp")
                    nc.vector.tensor_tensor(
                        out=rtmp[:], in0=summ[:], in1=recip0[:],
                        op=mybir.AluOpType.mult,
                    )
                    nc.vector.tensor_scalar(
                        out=rtmp[:], in0=rtmp[:], scalar1=2.0, scalar2=-1.0,
                        op0=mybir.AluOpType.subtract, op1=mybir.AluOpType.mult,
                    )
                    recip = sb.tile([BL, 1], F32, tag="recip")
                    nc.vector.tensor_tensor(
                        out=recip[:], in0=recip0[:], in1=rtmp[:],
                        op=mybir.AluOpType.mult,
                    )
                    en = sb.tile([BL, L], F32, tag="en")
                    nc.vector.tensor_scalar(
                        out=en[:], in0=em_[:], scalar1=recip[:], scalar2=None,
                        op0=mybir.AluOpType.mult,
                    )
                    # alphaT diag writes: abig[p, lc, s, s] = enT * vlocT
                    for lc in range(2):
                        eT_ps = ps.tile([P, BL], F32, space="PSUM", tag="tp", bufs=4)
                        nc.tensor.transpose(
                            out=eT_ps[:], in_=en[:, lc * P : (lc + 1) * P],
                            identity=ident[0:BL, 0:BL],
                        )
                        nc.vector.tensor_tensor(
                            out=_diag_ap(abig[:], lc, BL),
                            in0=eT_ps[:], in1=vlocT[:, lc, :],
                            op=mybir.AluOpType.mult,
                        )
                    # attention: accumulate into acc
                    for c in ([] if skip_attn else range(nch)):
                        slot, lc = (c, 0) if c < BL else (c - BL, 1)
                        r = chunk_rows[c]
                        blk = _block_ap(abig[:], lc, slot, BL)
                        blk = AP(blk.tensor, blk.offset, [[blk.ap[0][0], r], blk.ap[1]])
                        nc.tensor.matmul(
                            out=acc[:, 0:D],
                            lhsT=blk,
                            rhs=em[0:r, c, 0:D],
                            start=False, stop=(c == nch - 1), skip_group_check=True,
                        )
                    # vec = acc + b_lin
                    vec = sb.tile([BL, D], F32, tag="vec")
                    nc.vector.tensor_tensor(
                        out=vec[:], in0=acc[:, 0:D], in1=blin[:], op=mybir.AluOpType.add
                    )
                    if debug:
                        nc.sync.dma_start(dbg[f"d_vec{h}"][:], vec[:])
                        if h == 0:
                            nc.sync.dma_start(dbg["d_scor"][:], scor[:])
                            nc.sync.dma_start(dbg["d_en"][:], en[:])
                            nc.sync.dma_start(dbg["d_summ"][:], summ[:])

                # ---- output ----
                vecT = sb.tile([P, 3, BL], F32, tag="vecT")
                for ch in range(3):
                    k = min(P, D - ch * P)
                    vT = ps.tile([P, BL], F32, space="PSUM", tag="tp", bufs=4)
                    nc.tensor.transpose(
                        out=vT[0:k, :],
                        in_=vec[:, ch * P : ch * P + k],
                        identity=ident[0:BL, 0:BL],
                    )
                    nc.vector.tensor_copy(vecT[0:k, ch, :], vT[0:k, :])
                lg_ps = ps.tile([BL, 8], F32, space="PSUM", tag="tp", bufs=4)
                for ch in range(3):
                    k = min(P, D - ch * P)
                    nc.tensor.matmul(
                        out=lg_ps[:],
                        lhsT=vecT[0:k, ch, :],
                        rhs=wout[0:k, ch, :],
                        start=(ch == 0), stop=(ch == 2), skip_group_check=True,
                    )
                logits = sb.tile([BL, N_CLASSES], F32, tag="logits")
                nc.vector.tensor_tensor(
                    out=logits[:], in0=lg_ps[:, 0:N_CLASSES], in1=bout[:],
                    op=mybir.AluOpType.add,
                )
                nc.sync.dma_start(do_out[:], logits[:])

    nc.finalize()
    return nc


def prep_inputs(embeddings, w_lin, b_lin, w_att, b_att, w_out, b_out,
                context_ids, context_len, target_ids, target_len, target_loc):
    """Host-side prep: batch assignment, packed indices/weights, in_maps."""
    embeddings = np.ascontiguousarray(embeddings, np.float32)
    w_lin = np.asarray(w_lin, np.float32)
    b_lin = np.asarray(b_lin, np.float32)
    w_att = np.asarray(w_att, np.float32)
    b_att = float(np.asarray(b_att))
    w_out = np.asarray(w_out, np.float32)
    b_out = np.asarray(b_out, np.float32)
    context_ids = np.asarray(context_ids, np.int64)
    context_len = np.asarray(context_len, np.int64)
    target_ids = np.asarray(target_ids, np.int64)
    target_len = np.asarray(target_len, np.int64)
    target_loc = np.asarray(target_loc, np.int64)

    # sort descending by len, deal round-robin: slot i core k <- order[i*8+k]
    order = np.argsort(-context_len, kind="stable")
    assign = order.reshape(BL, NCORES)  # [slot, core]
    slot_max = context_len[assign].max(axis=1)  # [BL]
    nchunks = np.where(slot_max > P, 2, 1)
    n2 = int((nchunks == 2).sum())
    nch = BL + n2

    # augmented embedding table (col 300..303 pad) + host score-gather table
    ewa = np.concatenate([embeddings @ w_att[:D], np.zeros(1, np.float32)])
    emb_aug = np.zeros((VOCAB + 1, W), np.float32)
    emb_aug[:VOCAB, :D] = embeddings
    # per-chunk gathered row counts from per-slot max lens
    chunk_rows = []
    for c in range(BL + n2):
        slot, lc = (c, 0) if c < BL else (c - BL, 1)
        chunk_rows.append(int(min(max(int(slot_max[slot]) - lc * P, 1), P)))

    # packed weights (chunked on d, [128, 3, ...])
    def pack_d(x):  # x [300, ...] -> [128, 3, ...]
        out = np.zeros((P, 3) + x.shape[1:], np.float32)
        for ch in range(3):
            k = min(P, D - ch * P)
            out[:k, ch] = x[ch * P : ch * P + k]
        return out

    wlin_pk = pack_d(np.concatenate(
        [w_lin, w_att[D:, None], np.zeros((D, 3), np.float32)], axis=1))
    wout_pk = pack_d(np.concatenate(
        [w_out, np.zeros((D, 8 - N_CLASSES), np.float32)], axis=1))
    blin_rep = np.tile(b_lin[None, :], (BL, 1))
    bout_rep = np.tile(b_out[None, :], (BL, 1))

    in_maps = []
    for k in range(NCORES):
        bidx = assign[:, k]  # batch row per slot
        lens = context_len[bidx]
        locs = target_loc[bidx]
        tlens = target_len[bidx]

        idx_ctx = np.full((P, nch), VOCAB, np.int32)
        for c in range(nch):
            slot, lc = (c, 0) if c < BL else (c - BL, 1)
            ln = int(lens[slot])
            base = lc * P
            nvalid = min(max(ln - base, 0), P)
            if nvalid > 0:
                idx_ctx[:nvalid, c] = context_ids[bidx[slot], base : base + nvalid]
        # host-side hop-invariant score term: s0 = ewa[ids] masked by len
        ids_m = np.where(
            np.arange(L)[None, :] < lens[:, None], context_ids[bidx], VOCAB
        )
        s0_host = ewa[ids_m].astype(np.float32)

        idx_tgt = np.full((P, 2), VOCAB, np.int32)
        for ch in range(2):
            for j in range(16):
                slot = ch * 16 + j
                tl = int(tlens[slot])
                idx_tgt[j * 8 : j * 8 + tl, ch] = target_ids[bidx[slot], :tl]

        tw = np.zeros((P, 2, BL), np.float32)
        for ch in range(2):
            for j in range(16):
                tw[j * 8 : (j + 1) * 8, ch, ch * 16 + j] = 1.0 / float(tlens[ch * 16 + j])

        aux = np.stack(
            [
                lens.astype(np.float32),
                locs.astype(np.float32),
                (L - lens).astype(np.float32),
                1.0 / lens.astype(np.float32),
            ],
            axis=1,
        )
        in_maps.append(
            {
                "emb": emb_aug,
                "idx_ctx": idx_ctx,
                "idx_tgt": idx_tgt,
                "tw": tw,
                "wlin": wlin_pk,
                "wout": wout_pk,
                "blin": blin_rep,
                "bout": bout_rep,
                "aux": aux,
                "s0i": s0_host,
            }
        )
    return in_maps, assign, n2, b_att, chunk_rows


def kernel(**inputs) -> np.ndarray:
    in_maps, assign, n2, b_att, chunk_rows = prep_inputs(**inputs)
    nc = build_program(n2, b_att, chunk_rows, nq=4)
    res = run_bass_kernel_spmd(nc, in_maps, core_ids=list(range(NCORES)))
    out = np.zeros((B, N_CLASSES), np.float32)
    for k in range(NCORES):
        out[assign[:, k]] = res.results[k]["logits"]
    return out


# revision 14
# speedup vs baseline: 2.1345x; 1.7491x over previous
"""MemNet (6-hop memory network) Trainium2 kernel — 8-core data parallel.

Strategy:
  - Shard batch (B=256) across 8 cores, 32 slots each. Host sorts batch rows
    by context_len descending and deals them round-robin so the 8 cores have
    near-identical length profiles; the program is specialized to the per-slot
    max chunk count (1 or 2 l-chunks of 128).
  - Embedding rows are fetched with indirect-DMA row gathers from a host-
    augmented table [VOCAB+1, 304]: cols 0..299 = embedding, col 300 =
    emb @ w_att[:D] (the hop-invariant score term), row VOCAB = zeros
    (out-of-length ids redirect there, keeping everything NaN-free and exact).
  - Per hop: linear/proj matmuls from a transposed vec; scores via fused
    ACT ops (tanh/exp with per-partition bias, accum row-sum); softmax
    denominator corrected for the uniform masked tail; attention as per-slot
    thin matmuls with block-diagonal masked lhsT accumulating straight into
    the linear-out PSUM tile. v_loc is folded into the attention weights.
  - All matmul operands are float32r (~14 mantissa bits) for 1 cycle/row.
"""

import sys

sys.path.insert(0, "/opt/trn_rl_repo")

import numpy as np

import concourse.mybir as mybir
import concourse.tile as tile
from concourse.bacc import Bacc
from concourse.bass import AP, IndirectOffsetOnAxis
from concourse.bass_utils import run_bass_kernel_spmd
from concourse.masks import make_identity

VOCAB, D, L, LT, B = 50000, 300, 256, 8, 256
N_HOPS, N_CLASSES = 6, 3
NCORES = 8
BL = B // NCORES  # 32 slots per core
W = 304  # augmented row width: 300 emb + 1 ewa + 3 pad
P = 128
F32 = mybir.dt.float32
F32R = mybir.dt.float32r
I32 = mybir.dt.int32


def _diag_ap(t_ap, lc, n):
    """[128, n] stride-(n+1) diagonal view into block lc of [128, 2, n, n]."""
    return AP(t_ap.tensor, t_ap.offset + lc * n * n, [t_ap.ap[0], [n + 1, n]])


def _block_ap(t_ap, lc, slot, n):
    """[128, n] block (lc, slot) of [128, 2, n, n]."""
    return AP(
        t_ap.tensor, t_ap.offset + (lc * n + slot) * n, [t_ap.ap[0], [1, n]]
    )


def build_program(n2, b_att, chunk_rows, reps=1, debug=False, only_gather=False, skip_attn=False, nq=4):
    """Build the SPMD program. n2 = number of slots with 2 l-chunks
    (slots 0..n2-1), b_att = attention bias baked in, chunk_rows[c] = rows
    gathered/streamed for context chunk c (1..128, from per-slot max len)."""
    nch = BL + n2  # total context gather chunks
    nc = Bacc("TRN2", target_bir_lowering=False, debug=False, num_devices=NCORES,
              num_swdge_queues=nq)

    dt_emb = nc.dram_tensor("emb", (VOCAB + 1, W), F32, kind="ExternalInput").ap()
    dt_idx = nc.dram_tensor("idx_ctx", (P, nch), I32, kind="ExternalInput").ap()
    dt_idxt = nc.dram_tensor("idx_tgt", (P, 2), I32, kind="ExternalInput").ap()
    dt_tw = nc.dram_tensor("tw", (P, 2, BL), F32, kind="ExternalInput").ap()
    dt_wlin = nc.dram_tensor("wlin", (P, 3, D + 4), F32, kind="ExternalInput").ap()
    dt_wout = nc.dram_tensor("wout", (P, 3, 8), F32, kind="ExternalInput").ap()
    dt_blin = nc.dram_tensor("blin", (BL, D), F32, kind="ExternalInput").ap()
    dt_bout = nc.dram_tensor("bout", (BL, N_CLASSES), F32, kind="ExternalInput").ap()
    # aux cols: 0=len_f, 1=loc_f, 2=nlen_f(256-len), 3=rlen_f(1/len)
    dt_aux = nc.dram_tensor("aux", (BL, 4), F32, kind="ExternalInput").ap()
    dt_s0 = nc.dram_tensor("s0i", (BL, L), F32, kind="ExternalInput").ap()
    do_out = nc.dram_tensor("logits", (BL, N_CLASSES), F32, kind="ExternalOutput").ap()
    dbg = {}
    if debug:
        for nm, shp in (
            [("d_vaspect", (BL, D)), ("d_s0", (BL, L)), ("d_vloc", (BL, L)),
             ("d_scor", (BL, L)), ("d_en", (BL, L)), ("d_summ", (BL, 1))]
            + [(f"d_vec{h}", (BL, D)) for h in range(N_HOPS)]
        ):
            dbg[nm] = nc.dram_tensor(nm, shp, F32, kind="ExternalOutput").ap()

    with tile.TileContext(nc) as tc:
        with tc.tile_pool(name="sb", bufs=1) as sb, tc.tile_pool(
            name="ps", bufs=1, space="PSUM"
        ) as ps:
            # ---- persistent weights / constants (loaded once) ----
            ident = sb.tile([P, P], F32)
            make_identity(nc, ident[:])
            wlin = sb.tile([P, 3, D + 4], F32R)
            nc.gpsimd.dma_start(wlin[:], dt_wlin[:].bitcast(F32R))
            wout = sb.tile([P, 3, 8], F32)
            nc.sync.dma_start(wout[:], dt_wout[:])
            tw = sb.tile([P, 2, BL], F32)
            nc.sync.dma_start(tw[:], dt_tw[:])
            blin = sb.tile([BL, D], F32)
            nc.sync.dma_start(blin[:], dt_blin[:])
            bout = sb.tile([BL, N_CLASSES], F32)
            nc.sync.dma_start(bout[:], dt_bout[:])
            aux = sb.tile([BL, 4], F32)
            nc.sync.dma_start(aux[:], dt_aux[:])
            idx = sb.tile([P, nch], I32)
            nc.sync.dma_start(idx[:], dt_idx[:])
            idxt = sb.tile([P, 2], I32)
            nc.sync.dma_start(idxt[:], dt_idxt[:])
            s0 = sb.tile([BL, L], F32)
            nc.sync.dma_start(s0[:], dt_s0[:])
            neg1 = sb.tile([BL, 1], F32)
            nc.vector.memset(neg1[:], -1.0)
            zero128 = nc.const_aps.tensor(0.0, [P, 1])

            # iota / mask / vloc precompute (depends only on aux)
            iota_f = sb.tile([BL, L], F32)
            nc.gpsimd.iota(
                iota_f[:],
                pattern=[[1, L]],
                base=0,
                channel_multiplier=0,
                allow_small_or_imprecise_dtypes=True,
            )
            mask = sb.tile([BL, L], F32)
            nc.vector.tensor_scalar(
                out=mask[:], in0=iota_f[:], scalar1=aux[:, 0:1], scalar2=None,
                op0=mybir.AluOpType.is_lt,
            )
            dist = sb.tile([BL, L], F32)
            nc.vector.tensor_scalar(
                out=dist[:], in0=iota_f[:], scalar1=aux[:, 1:2], scalar2=None,
                op0=mybir.AluOpType.subtract,
            )
            nc.scalar.activation(dist[:], dist[:], mybir.ActivationFunctionType.Abs)
            vraw = sb.tile([BL, L], F32)
            # vraw = 1 - dist * rlen  (Copy: out = in*scale + bias)
            negr = sb.tile([BL, 1], F32)
            nc.vector.tensor_scalar(
                out=negr[:], in0=aux[:, 3:4], scalar1=-1.0, scalar2=None,
                op0=mybir.AluOpType.mult,
            )
            nc.scalar.activation(
                vraw[:], dist[:], mybir.ActivationFunctionType.Copy,
                bias=1.0, scale=negr[:],
            )
            vloc = sb.tile([BL, L], F32)
            nc.vector.tensor_tensor(
                out=vloc[:], in0=vraw[:], in1=mask[:], op=mybir.AluOpType.mult
            )
            # vlocT [128, 2, 32]
            vlocT = sb.tile([P, 2, BL], F32)
            for lc in range(2):
                vT_ps = ps.tile([P, BL], F32, space="PSUM", tag="tp", bufs=4)
                nc.tensor.transpose(
                    out=vT_ps[:], in_=vloc[:, lc * P : (lc + 1) * P],
                    identity=ident[0:BL, 0:BL],
                )
                nc.vector.tensor_copy(vlocT[:, lc, :], vT_ps[:])
            if debug:
                nc.sync.dma_start(dbg["d_vloc"][:], vloc[:])

            # big zeroed block-diag alpha tile [128, 2, 32, 32]
            abig = sb.tile([P, 2, BL, BL], F32R)
            abig_flat = AP(
                abig[:].tensor, abig[:].offset, [abig[:].ap[0], [1, 2 * BL * BL]]
            )
            nc.vector.tensor_copy(abig_flat, zero128.to_broadcast([P, 2 * BL * BL]))

            for rep in range(reps):
                # ---- gathers ----
                emt = sb.tile([P, 2, W], F32, tag="emt")
                for ch in range(2):
                    gi = nc.gpsimd.indirect_dma_start(
                        out=emt[:, ch, :],
                        out_offset=None,
                        in_=dt_emb[:],
                        in_offset=IndirectOffsetOnAxis(ap=idxt[:, ch : ch + 1], axis=0),
                    )
                    if nq > 1:
                        gi.ins.queue = f"qPoolDynamic{ch % nq or ''}"
                em = sb.tile([P, nch, W], F32R, tag="em")
                for c in range(nch):
                    r = chunk_rows[c]
                    gi = nc.gpsimd.indirect_dma_start(
                        out=em[0:r, c, :],
                        out_offset=None,
                        in_=dt_emb[:].bitcast(F32R),
                        in_offset=IndirectOffsetOnAxis(ap=idx[0:r, c : c + 1], axis=0),
                    )
                    if nq > 1:
                        gi.ins.queue = f"qPoolDynamic{c % nq or ''}"

                if only_gather:
                    # consume gathers so they aren't dead-code'd: one reduce
                    gsink = sb.tile([P, 1], F32, tag="gsink")
                    nc.vector.reduce_sum(
                        out=gsink[:], in_=em[:, :, 0].bitcast(F32),
                        axis=mybir.AxisListType.X,
                    )
                    nc.sync.dma_start(do_out[0:1, 0:1], gsink[0:1, :])
                    continue
                # ---- v_aspect -> vec ----
                vec = sb.tile([BL, D], F32, tag="vec")
                va_ps = ps.tile([BL, D], F32, space="PSUM", tag="tp", bufs=4)
                for ch in range(2):
                    nc.tensor.matmul(
                        out=va_ps[:],
                        lhsT=tw[:, ch, :],
                        rhs=emt[:, ch, 0:D],
                        start=(ch == 0), stop=(ch == 1), skip_group_check=True,
                    )
                nc.vector.tensor_copy(vec[:], va_ps[:])
                if debug:
                    nc.sync.dma_start(dbg["d_vaspect"][:], vec[:])

                if debug:
                    nc.sync.dma_start(dbg["d_s0"][:], s0[:])
                s0v = sb.tile([BL, L], F32, tag="s0v")
                nc.vector.tensor_tensor(
                    out=s0v[:], in0=s0[:], in1=vloc[:], op=mybir.AluOpType.mult
                )

                # ---- hops ----
                for h in range(N_HOPS):
                    # vecT [128, 3, 32]
                    vecT = sb.tile([P, 3, BL], F32R, tag="vecT")
                    for ch in range(3):
                        k = min(P, D - ch * P)
                        vT = ps.tile([P, BL], F32, space="PSUM", tag="tp", bufs=4)
                        nc.tensor.transpose(
                            out=vT[0:k, :],
                            in_=vec[:, ch * P : ch * P + k],
                            identity=ident[0:BL, 0:BL],
                        )
                        nc.vector.tensor_copy(vecT[0:k, ch, :], vT[0:k, :])
                    # linear_out (+proj in col 300) into acc psum
                    acc = ps.tile([BL, D + 4], F32, space="PSUM", tag="acc", bufs=2)
                    for ch in range(3):
                        k = min(P, D - ch * P)
                        nc.tensor.matmul(
                            out=acc[:],
                            lhsT=vecT[0:k, ch, :],
                            rhs=wlin[0:k, ch, :],
                            start=(ch == 0), stop=False, skip_group_check=True,
                        )
                    projb = sb.tile([BL, 1], F32, tag="projb")
                    nc.scalar.activation(
                        projb[:], acc[:, D : D + 1], mybir.ActivationFunctionType.Copy,
                        bias=float(b_att),
                    )
                    # scores -> e (exp(tanh(s0v + projb) - 1)), accum row sums
                    scor = sb.tile([BL, L], F32, tag="scor")
                    nc.scalar.activation(
                        scor[:], s0v[:], mybir.ActivationFunctionType.Tanh,
                        bias=projb[:],
                    )
                    e = sb.tile([BL, L], F32, tag="e")
                    nc.scalar.activation(
                        e[:], scor[:], mybir.ActivationFunctionType.Exp,
                        bias=neg1[:],
                    )
                    em_ = sb.tile([BL, L], F32, tag="em_")
                    nc.vector.tensor_tensor(
                        out=em_[:], in0=e[:], in1=mask[:], op=mybir.AluOpType.mult
                    )
                    summ = sb.tile([BL, 1], F32, tag="summ")
                    nc.vector.reduce_sum(
                        out=summ[:], in_=em_[:], axis=mybir.AxisListType.X
                    )
                    recip0 = sb.tile([BL, 1], F32, tag="recip0")
                    nc.vector.reciprocal(recip0[:], summ[:])
                    rtmp = sb.tile([BL, 1], F32, tag="rtmp")
                    nc.vector.tensor_tensor(
                        out=rtmp[:], in0=summ[:], in1=recip0[:],
                        op=mybir.AluOpType.mult,
                    )
                    nc.vector.tensor_scalar(
                        out=rtmp[:], in0=rtmp[:], scalar1=2.0, scalar2=-1.0,
                        op0=mybir.AluOpType.subtract, op1=mybir.AluOpType.mult,
                    )
                    recip = sb.tile([BL, 1], F32, tag="recip")
                    nc.vector.tensor_tensor(
                        out=recip[:], in0=recip0[:], in1=rtmp[:],
                        op=mybir.AluOpType.mult,
                    )
                    en = sb.tile([BL, L], F32, tag="en")
                    nc.vector.tensor_scalar(
                        out=en[:], in0=em_[:], scalar1=recip[:], scalar2=None,
                        op0=mybir.AluOpType.mult,
                    )
                    # alphaT diag writes: abig[p, lc, s, s] = enT * vlocT
                    for lc in range(2):
                        eT_ps = ps.tile([P, BL], F32, space="PSUM", tag="tp", bufs=4)
                        nc.tensor.transpose(
                            out=eT_ps[:], in_=en[:, lc * P : (lc + 1) * P],
                            identity=ident[0:BL, 0:BL],
                        )
                        nc.vector.tensor_tensor(
                            out=_diag_ap(abig[:], lc, BL),
                            in0=eT_ps[:], in1=vlocT[:, lc, :],
                            op=mybir.AluOpType.mult,
                        )
                    # attention: accumulate into acc
                    for c in ([] if skip_attn else range(nch)):
                        slot, lc = (c, 0) if c < BL else (c - BL, 1)
                        r = chunk_rows[c]
                        blk = _block_ap(abig[:], lc, slot, BL)
                        blk = AP(blk.tensor, blk.offset, [[blk.ap[0][0], r], blk.ap[1]])
                        nc.tensor.matmul(
                            out=acc[:, 0:D],
                            lhsT=blk,
                            rhs=em[0:r, c, 0:D],
                            start=False, stop=(c == nch - 1), skip_group_check=True,
                        )
                    # vec = acc + b_lin
                    vec = sb.tile([BL, D], F32, tag="vec")
                    nc.vector.tensor_tensor(
                        out=vec[:], in0=acc[:, 0:D], in1=blin[:], op=mybir.AluOpType.add
                    )
                    if debug:
                        nc.sync.dma_start(dbg[f"d_vec{h}"][:], vec[:])
                        if h == 0:
                            nc.sync.dma_start(dbg["d_scor"][:], scor[:])
                            nc.sync.dma_start(dbg["d_en"][:], en[:])
                            nc.sync.dma_start(dbg["d_summ"][:], summ[:])

                # ---- output ----
                vecT = sb.tile([P, 3, BL], F32, tag="vecT")
                for ch in range(3):
                    k = min(P, D - ch * P)
                    vT = ps.tile([P, BL], F32, space="PSUM", tag="tp", bufs=4)
                    nc.tensor.transpose(
                        out=vT[0:k, :],
                        in_=vec[:, ch * P : ch * P + k],
                        identity=ident[0:BL, 0:BL],
                    )
                    nc.vector.tensor_copy(vecT[0:k, ch, :], vT[0:k, :])
                lg_ps = ps.tile([BL, 8], F32, space="PSUM", tag="tp", bufs=4)
                for ch in range(3):
                    k = min(P, D - ch * P)
                    nc.tensor.matmul(
                        out=lg_ps[:],
                        lhsT=vecT[0:k, ch, :],
                        rhs=wout[0:k, ch, :],
                        start=(ch == 0), stop=(ch == 2), skip_group_check=True,
                    )
                logits = sb.tile([BL, N_CLASSES], F32, tag="logits")
                nc.vector.tensor_tensor(
                    out=logits[:], in0=lg_ps[:, 0:N_CLASSES], in1=bout[:],
                    op=mybir.AluOpType.add,
                )
                nc.sync.dma_start(do_out[:], logits[:])

    nc.finalize()
    return nc


def prep_inputs(embeddings, w_lin, b_lin, w_att, b_att, w_out, b_out,
                context_ids, context_len, target_ids, target_len, target_loc):
    """Host-side prep: batch assignment, packed indices/weights, in_maps."""
    embeddings = np.ascontiguousarray(embeddings, np.float32)
    w_lin = np.asarray(w_lin, np.float32)
    b_lin = np.asarray(b_lin, np.float32)
    w_att = np.asarray(w_att, np.float32)
    b_att = float(np.asarray(b_att))
    w_out = np.asarray(w_out, np.float32)
    b_out = np.asarray(b_out, np.float32)
    context_ids = np.asarray(context_ids, np.int64)
    context_len = np.asarray(context_len, np.int64)
    target_ids = np.asarray(target_ids, np.int64)
    target_len = np.asarray(target_len, np.int64)
    target_loc = np.asarray(target_loc, np.int64)

    # sort descending by len, deal round-robin: slot i core k <- order[i*8+k]
    order = np.argsort(-context_len, kind="stable")
    assign = order.reshape(BL, NCORES)  # [slot, core]
    slot_max = context_len[assign].max(axis=1)  # [BL]
    nchunks = np.where(slot_max > P, 2, 1)
    n2 = int((nchunks == 2).sum())
    nch = BL + n2

    # augmented embedding table (col 300..303 pad) + host score-gather table
    ewa = np.concatenate([embeddings @ w_att[:D], np.zeros(1, np.float32)])
    emb_aug = np.zeros((VOCAB + 1, W), np.float32)
    emb_aug[:VOCAB, :D] = embeddings
    # per-chunk gathered row counts from per-slot max lens
    chunk_rows = []
    for c in range(BL + n2):
        slot, lc = (c, 0) if c < BL else (c - BL, 1)
        chunk_rows.append(int(min(max(int(slot_max[slot]) - lc * P, 1), P)))

    # packed weights (chunked on d, [128, 3, ...])
    def pack_d(x):  # x [300, ...] -> [128, 3, ...]
        out = np.zeros((P, 3) + x.shape[1:], np.float32)
        for ch in range(3):
            k = min(P, D - ch * P)
            out[:k, ch] = x[ch * P : ch * P + k]
        return out

    wlin_pk = pack_d(np.concatenate(
        [w_lin, w_att[D:, None], np.zeros((D, 3), np.float32)], axis=1))
    wout_pk = pack_d(np.concatenate(
        [w_out, np.zeros((D, 8 - N_CLASSES), np.float32)], axis=1))
    blin_rep = np.tile(b_lin[None, :], (BL, 1))
    bout_rep = np.tile(b_out[None, :], (BL, 1))

    in_maps = []
    for k in range(NCORES):
        bidx = assign[:, k]  # batch row per slot
        lens = context_len[bidx]
        locs = target_loc[bidx]
        tlens = target_len[bidx]

        idx_ctx = np.full((P, nch), VOCAB, np.int32)
        for c in range(nch):
            slot, lc = (c, 0) if c < BL else (c - BL, 1)
            ln = int(lens[slot])
            base = lc * P
            nvalid = min(max(ln - base, 0), P)
            if nvalid > 0:
                idx_ctx[:nvalid, c] = context_ids[bidx[slot], base : base + nvalid]
        # host-side hop-invariant score term: s0 = ewa[ids] masked by len
        ids_m = np.where(
            np.arange(L)[None, :] < lens[:, None], context_ids[bidx], VOCAB
        )
        s0_host = ewa[ids_m].astype(np.float32)

        idx_tgt = np.full((P, 2), VOCAB, np.int32)
        for ch in range(2):
            for j in range(16):
                slot = ch * 16 + j
                tl = int(tlens[slot])
                idx_tgt[j * 8 : j * 8 + tl, ch] = target_ids[bidx[slot], :tl]

        tw = np.zeros((P, 2, BL), np.float32)
        for ch in range(2):
            for j in range(16):
                tw[j * 8 : (j + 1) * 8, ch, ch * 16 + j] = 1.0 / float(tlens[ch * 16 + j])

        aux = np.stack(
            [
                lens.astype(np.float32),
                locs.astype(np.float32),
                (L - lens).astype(np.float32),
                1.0 / lens.astype(np.float32),
            ],
            axis=1,
        )
        in_maps.append(
            {
                "emb": emb_aug,
                "idx_ctx": idx_ctx,
                "idx_tgt": idx_tgt,
                "tw": tw,
                "wlin": wlin_pk,
                "wout": wout_pk,
                "blin": blin_rep,
                "bout": bout_rep,
                "aux": aux,
                "s0i": s0_host,
            }
        )
    return in_maps, assign, n2, b_att, chunk_rows


def kernel(**inputs) -> np.ndarray:
    in_maps, assign, n2, b_att, chunk_rows = prep_inputs(**inputs)
    nc = build_program(n2, b_att, chunk_rows, nq=4)
    res = run_bass_kernel_spmd(nc, in_maps, core_ids=list(range(NCORES)))
    out = np.zeros((B, N_CLASSES), np.float32)
    for k in range(NCORES):
        out[assign[:, k]] = res.results[k]["logits"]
    return out
